# revision 10
# baseline (speedup 1.0000x reference)
"""DistogramHead Trainium2 kernel.

Math (reference):
    hi = x @ W1[:d]          # [N, d]
    hj = x @ W1[d:]          # [N, d]
    h[i,j,:] = gelu(hi[i] + hj[j] + b1)
    out[i,j,:] = h[i,j,:] @ W2 + b2      # [N, N, bins]

Sharding: i-axis split across 8 cores (64 rows each), weights/x replicated.
The host passes xT (= x transposed) and the per-core xiT slice so the device
needs no transposes; the device emits out as [pair, 2*64(e), 512(j)] and the
host transposes to [i, j, e] while unsharding.

Per-core layout strategy (d on partitions):
    hjT  [d,j]  = W1b^T x^T  (+ b1 folded in on PSUM->SBUF evac)   4 tiles [128,512]
    hiT  [d,i]  = W1a^T xi^T                                        4 tiles [128,64]
    per i:  h_in[d, j] = hjT + hiT[:,i]  (DVE tensor_scalar, per-partition scalar)
            gh = Gelu(h_in)              (ACT, large free-dim to amortize overhead)
            psum[e, j] += W2c^T gh_c     (PE, W2 chunk stationary, N=512 streaming)
    Two i's share one PSUM bank ([0:64] / [64:128] via col tile_position), so the
    DVE evacuation (+b2 fold) runs once per pair on a full [128,512] tile.
"""

from contextlib import ExitStack

import numpy as np

import concourse.bass as bass
import concourse.bacc as bacc
import concourse.tile as tile
from concourse import mybir

N = 512  # sequence length (pair grid is N x N)
D = 512  # d_model
BINS = 64
NCORES = 8
NI = N // NCORES  # i rows per core
F32 = mybir.dt.float32
PD = D // 128  # number of 128-wide d chunks (4)


def build_nc(n_i: int = NI, act_group: int = 2, act_fn=None):
    """Build the single-core Bass program (SPMD across 8 cores).

    act_group: number of i rows fused into one ACTIVATE instruction
    (free dim = act_group * 4 * 512). Must be even (pairs share PSUM banks).
    """
    assert n_i % act_group == 0 and act_group % 2 == 0
    if act_fn is None:
        act_fn = mybir.ActivationFunctionType.Gelu
    nc = bacc.Bacc("TRN2", target_bir_lowering=False, debug=False, num_devices=NCORES)

    xt_d = nc.dram_tensor("xt", [D, N], F32, kind="ExternalInput").ap()
    xit_d = nc.dram_tensor("xit", [D, n_i], F32, kind="ExternalInput").ap()
    w1_d = nc.dram_tensor("w1", [2 * D, D], F32, kind="ExternalInput").ap()
    w2_d = nc.dram_tensor("w2", [D, BINS], F32, kind="ExternalInput").ap()
    b1_d = nc.dram_tensor("b1", [D], F32, kind="ExternalInput").ap()
    b2_d = nc.dram_tensor("b2", [BINS], F32, kind="ExternalInput").ap()
    out_d = nc.dram_tensor("out", [n_i // 2, 128, N], F32, kind="ExternalOutput").ap()

    FD = act_group * PD * N  # free dim of one h buffer

    with tile.TileContext(nc) as tc:
        with (
            tc.tile_pool(name="const", bufs=1) as constp,
            tc.tile_pool(name="hbuf", bufs=3) as hpool,
            tc.tile_pool(name="gbuf", bufs=3) as gpool,
            tc.tile_pool(name="outsb", bufs=4) as outp,
        ):
            xt = [constp.tile([128, N], F32, tag=f"xt{c}", name=f"xt{c}") for c in range(PD)]
            xit = [constp.tile([128, n_i], F32, tag=f"xit{c}", name=f"xit{c}") for c in range(PD)]
            hjt = [constp.tile([128, N], F32, tag=f"hjt{c}", name=f"hjt{c}") for c in range(PD)]
            hit = [constp.tile([128, n_i], F32, tag=f"hit{c}", name=f"hit{c}") for c in range(PD)]
            w2_sb = [constp.tile([128, BINS], F32, tag=f"w2{c}", name=f"w2sb{c}") for c in range(PD)]
            b1c = constp.tile([128, PD], F32)
            b2dup = constp.tile([128, 1], F32)

            for c in range(PD):
                nc.gpsimd.dma_start(xt[c][:], xt_d[c * 128 : (c + 1) * 128, :])
                nc.gpsimd.dma_start(xit[c][:], xit_d[c * 128 : (c + 1) * 128, :])
                nc.gpsimd.dma_start(w2_sb[c][:], w2_d[c * 128 : (c + 1) * 128, :])
            nc.gpsimd.dma_start(b1c[:], b1_d.rearrange("(c p) -> p c", c=PD))
            b2col = b2_d.rearrange("(e one) -> e one", one=1)
            nc.gpsimd.dma_start(b2dup[0:64, :], b2col)
            nc.gpsimd.dma_start(b2dup[64:128, :], b2col)

            # ---- prologue: first-layer matmuls ----
            with (
                tc.tile_pool(name="pro_sb", bufs=1) as prop,
                tc.tile_pool(name="pro_ps", bufs=3, space="PSUM") as propsum,
            ):
                w1b_sb = [prop.tile([128, D], F32, tag=f"w1b{c}", name=f"w1bsb{c}") for c in range(PD)]
                w1a_sb = [prop.tile([128, D], F32, tag=f"w1a{c}", name=f"w1asb{c}") for c in range(PD)]
                for k in range(PD):
                    nc.gpsimd.dma_start(
                        w1a_sb[k][:], w1_d[k * 128 : (k + 1) * 128, :]
                    )
                    nc.gpsimd.dma_start(
                        w1b_sb[k][:], w1_d[D + k * 128 : D + (k + 1) * 128, :]
                    )

                # hjT[d, j] = sum_k W1b[k, d]^T x^T[k, j]; fold b1 on evac.
                for dc in range(PD):
                    pj = propsum.tile([128, N], F32, tag="pp", name="pj")
                    for k in range(PD):
                        nc.tensor.matmul(
                            pj[:],
                            w1b_sb[k][:, dc * 128 : (dc + 1) * 128],
                            xt[k][:],
                            start=(k == 0),
                            stop=(k == PD - 1),
                        )
                    nc.vector.tensor_scalar_add(hjt[dc][:], pj[:], b1c[:, dc : dc + 1])
                # hiT[d, i] = sum_k W1a[k, d]^T xi^T[k, i]
                for dc in range(PD):
                    pi = propsum.tile([128, n_i], F32, tag="pp", name="pi")
                    for k in range(PD):
                        nc.tensor.matmul(
                            pi[:],
                            w1a_sb[k][:, dc * 128 : (dc + 1) * 128],
                            xit[k][:],
                            start=(k == 0),
                            stop=(k == PD - 1),
                        )
                    nc.vector.tensor_copy(hit[dc][:], pi[:])

            # ---- main loop over i groups ----
            mm_ctx = ExitStack()
            psump = mm_ctx.enter_context(tc.tile_pool(name="mm", bufs=6, space="PSUM"))
            for g in range(n_i // act_group):
                hb = hpool.tile([128, FD], F32)
                for m in range(act_group):
                    i = g * act_group + m
                    for dc in range(PD):
                        lo = (m * PD + dc) * N
                        nc.vector.tensor_scalar_add(
                            hb[:, lo : lo + N], hjt[dc][:], hit[dc][:, i : i + 1]
                        )
                gb = gpool.tile([128, FD], F32)
                nc.scalar.activation(gb[:], hb[:], act_fn)

                for p in range(act_group // 2):
                    ps = psump.tile([128, N], F32)
                    for half in range(2):
                        m = 2 * p + half
                        for dc in range(PD):
                            lo = (m * PD + dc) * N
                            nc.tensor.matmul(
                                ps[half * 64 : half * 64 + 64, :],
                                w2_sb[dc][:],
                                gb[:, lo : lo + N],
                                start=(dc == 0),
                                stop=(dc == PD - 1),
                                tile_position=(0, half * 64),
                            )
                    ob = outp.tile([128, N], F32)
                    nc.vector.tensor_scalar_add(ob[:], ps[:], b2dup[:, 0:1])
                    pair = g * (act_group // 2) + p
                    nc.gpsimd.dma_start(out_d[pair], ob[:])
            mm_ctx.close()
    nc.compile()
    return nc


def _host_assemble(per_core_outs, n_i=NI):
    """[pair, 128, 512] per core -> full [1, N, N, BINS]."""
    blocks = []
    for o in per_core_outs:
        o = o.reshape(n_i // 2, 2, BINS, N)  # pair, half, e, j
        o = o.reshape(n_i, BINS, N).transpose(0, 2, 1)  # i, j, e
        blocks.append(o)
    return np.concatenate(blocks, axis=0)[None]


def make_in_maps(x, W1, b1, W2, b2, n_cores=NCORES, n_i=NI):
    x2 = np.asarray(x, dtype=np.float32).reshape(N, D)
    xt = np.ascontiguousarray(x2.T)  # [k, j]
    w1 = np.ascontiguousarray(W1, dtype=np.float32)
    w2 = np.ascontiguousarray(W2, dtype=np.float32)
    b1 = np.ascontiguousarray(b1, dtype=np.float32)
    b2 = np.ascontiguousarray(b2, dtype=np.float32)
    maps = []
    for c in range(n_cores):
        maps.append(
            {
                "xt": xt,
                "xit": np.ascontiguousarray(xt[:, c * n_i : (c + 1) * n_i]),
                "w1": w1,
                "w2": w2,
                "b1": b1,
                "b2": b2,
            }
        )
    return maps


_NC_CACHE = {}


def kernel(x, W1, b1, W2, b2):
    from concourse.bass_utils import run_bass_kernel_spmd

    if "nc" not in _NC_CACHE:
        _NC_CACHE["nc"] = build_nc()
    nc = _NC_CACHE["nc"]
    in_maps = make_in_maps(x, W1, b1, W2, b2)
    res = run_bass_kernel_spmd(nc, in_maps, core_ids=list(range(NCORES)))
    outs = [res.results[c]["out"] for c in range(NCORES)]
    return _host_assemble(outs).astype(np.float32)
